# revision 33
# baseline (speedup 1.0000x reference)
"""Self-contained Trainium2 Bass kernel for a Transformer encoder layer.

Reference computation (fp32):
    q,k,v = x@wq, x@wk, x@wv          (per-head split, DK=64)
    attn  = softmax(q@k^T/sqrt(DK) + mask_bias) @ v
    x1    = LN(x + attn@wo) * g1 + be1
    out   = LN(x1 + relu(x1@w1 + b1)@w2 + b2) * g2 + be2

Sharding: pure data-parallel over (batch, seq). 8 cores; core c owns batch
c//4 and a 1024-row query shard (c%4). K/V projections for the full batch
are computed redundantly on each core (no collectives).

Key implementation choices (v2):
  - All matmul operands bf16 (weights host-cast): enables Fast Weight Load
    (4x faster LDWEIGHTS vs fp32r) at identical PE streaming rate; psum
    accumulation stays fp32.
  - K^T / V / Q^T stay SBUF-resident (no DRAM spill, no phase barrier).
  - softmax exp runs mostly on the Vector engine via the Schraudolph bit
    trick (int16(s*A+B) bitcast bf16, ~+-3% relative, largely cancelling
    between numerator and rowsum), a fraction on ACT (true Exp) for load
    balance. Mask bias folds into the trick's additive constant; fully
    masked scores saturate the int16 convert to -32768 -> bf16 -0.0.
  - softmax rowsum rides the ctx matmul as a ones column of V (M=65);
    normalization = ACT row copy + gpsimd partition_broadcast +
    reciprocal_approx_fast + one DVE multiply, all on-chip.
  - 1-deep software pipelines (scores(kt+1) ahead of ctx(kt), transposes
    (sl+1) ahead of proj MMs(sl)) keep the PE from stalling on exp/copies.
"""

import os
import sys

import numpy as np

if os.path.isdir("/opt/trn_rl_repo") and "/opt/trn_rl_repo" not in sys.path:
    sys.path.insert(0, "/opt/trn_rl_repo")

import ml_dtypes

import concourse.bacc as bacc
import concourse.bass as bass
import concourse.tile as tile
from concourse import mybir
from concourse.bass_utils import run_bass_kernel_spmd
from concourse.masks import make_identity

B, S, D, H, DK = 2, 4096, 512, 8, 64
DFF = 2048
EPS = 1e-5
N_CORES = 8
SHARD = S // 4  # 1024 query rows per core
F32 = mybir.dt.float32
BF16 = mybir.dt.bfloat16
I16 = mybir.dt.int16
I8 = mybir.dt.int8
FP8 = mybir.dt.float8e4
AF = mybir.ActivationFunctionType
ALU = mybir.AluOpType
PM_DR = mybir.MatmulPerfMode.DoubleRow

NSLICE = S // 512        # 8 column slices of x^T
NQSL = SHARD // 512      # 2 slices for the Q shard
NPAIR = H // 2           # 4 head pairs
NKT = S // 128           # 32 key tiles
NQT = SHARD // 128       # 8 query tiles in the shard
NDC = D // 128           # 4 contraction chunks of D
NFC = DFF // 128         # 16 chunks of DFF

# Schraudolph fast-exp constants, fp8e4m3 output via int8 bit trick:
# p~ = bitcast_fp8(int8(max(s*EXA + bias, 0))).  K8 folds a 2^-4 scale
# into every p (softmax is scale-invariant; the rowsum rides along): the
# ACT true-exp path then peaks at e^8.03*2^-4 = 193, under the ~240
# ceiling where the hardware ACT->fp8 convert overflows to inf, and the
# DVE int8 trick peaks at t=117, well under the 127=NaN encoding
# regardless of convert rounding mode (no +0.5: the int8 convert was
# observed to round, unlike the truncating int16 convert).
LOG2E = 1.4426950408889634
EXC = 0.0303
K8 = -4.0
EXA = 8.0 * LOG2E
EXB = (7.0 - EXC) * 8.0 + 8.0 * K8
NKP = NKT // 2           # key-tile pairs (fp8 DoubleRow contracts 2 tiles)


def _build_program(apply_affine1, apply_affine2, apply_b2):
    nc = bacc.Bacc("TRN2", target_bir_lowering=False, debug=False,
                   num_devices=N_CORES)

    xf = nc.declare_dram_parameter("xf", [S, D], F32, isOutput=False)
    xq = nc.declare_dram_parameter("xq", [SHARD, D], F32, isOutput=False)
    mbt = nc.declare_dram_parameter("mbt", [S], F32, isOutput=False)  # trick bias
    mbr = nc.declare_dram_parameter("mbr", [S], F32, isOutput=False)  # raw bias
    wq = nc.declare_dram_parameter("wq", [D, D], BF16, isOutput=False)
    wk = nc.declare_dram_parameter("wk", [D, D], BF16, isOutput=False)
    wv = nc.declare_dram_parameter("wv", [D, D], BF16, isOutput=False)
    wo = nc.declare_dram_parameter("wo", [D, D], BF16, isOutput=False)
    w1 = nc.declare_dram_parameter("w1", [D, DFF], BF16, isOutput=False)
    b1 = nc.declare_dram_parameter("b1", [DFF], F32, isOutput=False)
    w2 = nc.declare_dram_parameter("w2", [DFF, D], BF16, isOutput=False)
    b2 = nc.declare_dram_parameter("b2", [D], F32, isOutput=False)
    g1 = nc.declare_dram_parameter("g1", [D], F32, isOutput=False)
    be1 = nc.declare_dram_parameter("be1", [D], F32, isOutput=False)
    g2 = nc.declare_dram_parameter("g2", [D], F32, isOutput=False)
    be2 = nc.declare_dram_parameter("be2", [D], F32, isOutput=False)
    out = nc.declare_dram_parameter("out", [SHARD, D], F32, isOutput=True)

    def bcast_ap(vec, parts):
        a = vec if isinstance(vec, bass.AP) else vec.ap()
        ap_dims = [list(d) for d in a.ap]
        if len(ap_dims) > 1 and ap_dims[0][1] == 1:
            ap_dims = ap_dims[1:]
        return bass.AP(tensor=a.tensor, offset=a.offset,
                       ap=[[0, parts]] + ap_dims)

    import contextlib
    with tile.TileContext(nc, pool_alloc_mode="queue") as tc, \
         contextlib.ExitStack() as ctx:
        consts = ctx.enter_context(tc.tile_pool(name="consts", bufs=1))
        ident = consts.tile([128, 128], F32)
        make_identity(nc, ident)
        mbt_t = consts.tile([128, NKT], F32)
        nc.sync.dma_start(out=mbt_t, in_=mbt.ap().rearrange("(t p) -> p t", p=128))
        mbr_t = consts.tile([128, NKT], F32)
        nc.sync.dma_start(out=mbr_t, in_=mbr.ap().rearrange("(t p) -> p t", p=128))
        epst = consts.tile([128, 1], F32)
        nc.vector.memset(epst, EPS)

        # late-phase weights: tiles allocated here (pool stack order), DMAs
        # emitted inside phase 1 after the critical slice-0 loads
        wlate = ctx.enter_context(tc.tile_pool(name="wlate", bufs=1))
        wo_sb = wlate.tile([64, H, D], BF16)
        w1_sb = wlate.tile([128, NDC, DFF], BF16)
        w2_sb = wlate.tile([128, NFC, D], BF16)
        b1_sb = wlate.tile([128, NFC], F32)
        b2b = g1b = be1b = g2b = be2b = None
        if apply_b2:
            b2b = wlate.tile([128, D], F32)
        if apply_affine1:
            g1b = wlate.tile([128, D], F32)
            be1b = wlate.tile([128, D], F32)
        if apply_affine2:
            g2b = wlate.tile([128, D], F32)
            be2b = wlate.tile([128, D], F32)

        def emit_wlate_dmas():
            nc.sync.dma_start(out=wo_sb, in_=wo.ap().rearrange("(h p) n -> p h n", p=64))
            nc.sync.dma_start(out=w1_sb, in_=w1.ap().rearrange("(c p) n -> p c n", p=128))
            nc.sync.dma_start(out=w2_sb, in_=w2.ap().rearrange("(f p) n -> p f n", p=128))
            nc.sync.dma_start(out=b1_sb, in_=b1.ap().rearrange("(f p) -> p f", p=128))
            if apply_b2:
                nc.sync.dma_start(out=b2b, in_=bcast_ap(b2, 128))
            if apply_affine1:
                nc.sync.dma_start(out=g1b, in_=bcast_ap(g1, 128))
                nc.sync.dma_start(out=be1b, in_=bcast_ap(be1, 128))
            if apply_affine2:
                nc.sync.dma_start(out=g2b, in_=bcast_ap(g2, 128))
                nc.sync.dma_start(out=be2b, in_=bcast_ap(be2, 128))

        # normalized ctx^T per head — written in phase 2, read in phase 3;
        # allocated before the attn pool so pools release in stack order
        ln1 = ctx.enter_context(tc.tile_pool(name="ln1", bufs=1))
        cn = ln1.tile([64, H, SHARD], BF16)

        # ---- residents for phases 1-2 ---------------------------------
        es_attn = contextlib.ExitStack()
        attn_pool = es_attn.enter_context(tc.tile_pool(name="attn", bufs=1))
        kTt = attn_pool.tile([128, NPAIR, S], BF16)          # pair-packed K^T
        # V + ones col, fp8, kt-pair-interleaved for DoubleRow ctx matmuls;
        # +2 pad makes the DoubleRow Ko stride (H*(DK+2) = 528B) 16-aligned
        v1 = attn_pool.tile([128, NKP, 2, H, DK + 2], FP8)
        nc.vector.memset(v1[:, :, :, :, DK:DK + 1], 1.0)
        qT = attn_pool.tile([128, NPAIR, SHARD], BF16)       # pair-packed Q^T

        # ---- phase 1: projections -------------------------------------
        with tc.tile_pool(name="pw", bufs=1) as pw, \
             tc.tile_pool(name="p1s", bufs=4) as p1s, \
             tc.tile_pool(name="p1x", bufs=2) as p1x, \
             tc.tile_pool(name="p1p", bufs=3, space="PSUM") as p1p, \
             tc.tile_pool(name="p1tp", bufs=4, space="PSUM") as p1tp:
            wq_sb = pw.tile([128, NDC, D], BF16)
            nc.sync.dma_start(out=wq_sb, in_=wq.ap().rearrange("(c p) n -> p c n", p=128))
            wk_sb = pw.tile([128, NDC, D], BF16)
            nc.sync.dma_start(out=wk_sb, in_=wk.ap().rearrange("(c p) n -> p c n", p=128))
            wv_sb = pw.tile([128, NDC, D], BF16)
            nc.sync.dma_start(out=wv_sb, in_=wv.ap().rearrange("(c p) n -> p c n", p=128))

            def emit_transposes(src, sl):
                """DMA 512 rows of src and emit PE transposes + DVE copies.
                4 transposes land in one psum tile so each DVE copy moves
                [128, 512] (per-instruction overhead amortized)."""
                xT = p1x.tile([128, NDC, 512], BF16, tag="xTs")
                xts = []
                for m in range(4):
                    xt = p1s.tile([128, D], F32, tag="xload")
                    nc.sync.dma_start(out=xt, in_=src[sl * 512 + m * 128:
                                                      sl * 512 + (m + 1) * 128, :])
                    xts.append(xt)
                for c in range(NDC):
                    tp = p1tp.tile([128, 512], F32, tag="tpp")
                    for m in range(4):
                        nc.tensor.transpose(tp[:, m * 128:(m + 1) * 128],
                                            xts[m][:, c * 128:(c + 1) * 128], ident)
                    nc.vector.tensor_copy(out=xT[:, c, :], in_=tp)
                return xT

            def emit_proj(sl, xT):
                """K^T and V matmuls for full-batch slice sl."""
                for pr in range(NPAIR):
                    kp = p1p.tile([128, 512], F32, tag="kpsum")
                    for c in range(NDC):
                        nc.tensor.matmul(kp, wk_sb[:, c, pr * 128:(pr + 1) * 128],
                                         xT[:, c, :], start=(c == 0),
                                         stop=(c == NDC - 1))
                    nc.scalar.copy(out=kTt[:, pr, sl * 512:(sl + 1) * 512], in_=kp)
                for m in range(4):
                    vp = p1p.tile([128, 512], F32, tag="kpsum")
                    for c in range(NDC):
                        nc.tensor.matmul(vp, xT[:, c, m * 128:(m + 1) * 128],
                                         wv_sb[:, c, :], start=(c == 0),
                                         stop=(c == NDC - 1))
                    kt_ = sl * 4 + m
                    nc.vector.tensor_copy(
                        out=v1[:, kt_ // 2, kt_ % 2, :, 0:DK], in_=vp)

            def emit_q(qsl, xT):
                for pr in range(NPAIR):
                    qp = p1p.tile([128, 512], F32, tag="kpsum")
                    for c in range(NDC):
                        nc.tensor.matmul(qp, wq_sb[:, c, pr * 128:(pr + 1) * 128],
                                         xT[:, c, :], start=(c == 0),
                                         stop=(c == NDC - 1))
                    nc.scalar.copy(out=qT[:, pr, qsl * 512:(qsl + 1) * 512],
                                   in_=qp)

            # software pipeline: transposes one slice ahead of its MMs
            work = [("kv", sl) for sl in range(NSLICE)] + \
                   [("q", qsl) for qsl in range(NQSL)]
            xT_prev = None
            for i, (kind, sl) in enumerate(work):
                src = xf if kind == "kv" else xq
                xT_cur = emit_transposes(src, sl)
                if i == 2:
                    emit_wlate_dmas()  # after the first slices' loads
                if xT_prev is not None:
                    pk, psl, pxT = xT_prev
                    emit_proj(psl, pxT) if pk == "kv" else emit_q(psl, pxT)
                xT_prev = (kind, sl, xT_cur)
            pk, psl, pxT = xT_prev
            emit_proj(psl, pxT) if pk == "kv" else emit_q(psl, pxT)

        # ---- phase 2: attention ---------------------------------------
        with tc.tile_pool(name="ppool", bufs=6) as ppool, \
             tc.tile_pool(name="rpool", bufs=3) as rpool, \
             tc.tile_pool(name="spsum", bufs=2, space="PSUM") as spsum, \
             tc.tile_pool(name="cpsum", bufs=2, space="PSUM") as cpsum:
            for pr in range(NPAIR):
                cA = cpsum.tile([DK + 1, SHARD], F32, tag="ctx")
                cB = cpsum.tile([DK + 1, SHARD], F32, tag="ctx")
                # scores/exp stream per kt; fp8 DoubleRow ctx per kt-PAIR,
                # emitted 1.5 pairs behind so the ~1.2us exp never stalls
                # the PE.  pT tiles hold both kts of a pair interleaved.
                cur = {}
                hist = {}
                for kt in range(NKT + 2):
                    if kt < NKT:
                        j = kt % 2
                        for qh in range(NQSL):
                            if j == 0 and qh == 0:
                                pTn0 = ppool.tile([128, 2, SHARD], I8,
                                                  tag="pT0")
                                cur[0] = pTn0
                            if j == 0 and qh == 1:
                                pTn1 = ppool.tile([128, 2, SHARD], I8,
                                                  tag="pT1")
                                cur[1] = pTn1
                            pT = cur[qh]
                            # per-head 1-bank score tiles; each half exps on
                            # a different engine concurrently, so the psum
                            # frees ~2x sooner and 4 bufs pipeline 2 kts.
                            for hh in (0, 1):
                                lo, hi = hh * 64, hh * 64 + 64
                                sph = spsum.tile([128, 512], F32,
                                                 tag=f"sc{hh}")
                                nc.tensor.matmul(
                                    sph,
                                    kTt[lo:hi, pr, kt * 128:(kt + 1) * 128],
                                    qT[lo:hi, pr, qh * 512:(qh + 1) * 512],
                                    start=True, stop=True)
                                dst8 = pT[:, j, hh * 512:(hh + 1) * 512]
                                if hh == ((kt + qh) % 2):
                                    nc.scalar.activation(
                                        dst8.bitcast(FP8), sph, AF.Exp,
                                        bias=mbr_t[:, kt:kt + 1],
                                        scale=1.0 / EXA)
                                else:
                                    nc.vector.tensor_scalar(
                                        out=dst8, in0=sph,
                                        scalar1=mbt_t[:, kt:kt + 1],
                                        scalar2=0.0,
                                        op0=ALU.add, op1=ALU.max)
                        if j == 1:
                            hist[kt // 2] = (cur[0], cur[1])
                    if kt % 2 == 1 and kt >= 3:
                        ktp = (kt - 3) // 2
                        p0, p1 = hist.pop(ktp)
                        p08, p18 = p0.bitcast(FP8), p1.bitcast(FP8)
                        for hh, cps in ((0, cA), (1, cB)):
                            h = 2 * pr + hh
                            va = v1[:, ktp, 0, h, 0:DK + 1]
                            lhsT = bass.AP(
                                tensor=va.tensor, offset=va.offset,
                                ap=[list(va.ap[0]), [H * (DK + 2), 2],
                                    [1, DK + 1]])
                            for qh, p8 in ((0, p08), (1, p18)):
                                nc.tensor.matmul(
                                    cps[:, qh * 512:(qh + 1) * 512],
                                    lhsT,
                                    p8[:, :, hh * 512:(hh + 1) * 512],
                                    start=(ktp == 0), stop=(ktp == NKP - 1),
                                    perf_mode=PM_DR)
                # normalize: rowsum bcast -> approx recip -> multiply.
                # Pairs 0-2: stage to SBUF and multiply on gpsimd (keeps the
                # DVE free for exp).  Last pair: shortest-latency DVE path
                # straight from psum, so the phase-3 boundary gap stays
                # under the ~3.4us HAM re-throttle window.
                last = (pr == NPAIR - 1)
                for hh, cps in ((0, cA), (1, cB)):
                    h = 2 * pr + hh
                    # dedicated base-partition-0 tile for the broadcast:
                    # partition_broadcast reads absolute partition 0 on HW
                    rsr = rpool.tile([1, SHARD], F32, tag="rsr")
                    nc.scalar.copy(out=rsr, in_=cps[DK:DK + 1, :])
                    rb = rpool.tile([64, SHARD], F32, tag="rb")
                    nc.gpsimd.partition_broadcast(rb, rsr)
                    nc.vector.reciprocal_approx_fast(out=rb, in_=rb)
                    if last:
                        nc.vector.tensor_mul(out=cn[:, h, :],
                                             in0=cps[0:DK, :], in1=rb)
                    else:
                        stg = rpool.tile([DK + 1, SHARD], F32, tag="stg")
                        nc.scalar.copy(out=stg, in_=cps)
                        nc.gpsimd.tensor_mul(out=cn[:, h, :],
                                             in0=stg[0:DK, :], in1=rb)
        es_attn.close()  # free kTt + v1 + qT

        # ---- phases 3-5: wo+LN1, x1^T, FFN1, FFN2+LN2 -----------------
        # PE emission order: all wo matmuls (m0..7) -> transposes m0..3 ->
        # FFN1 qh0 -> transposes m4..7 -> FFN2 m0..3 -> FFN1 qh1 ->
        # FFN2 m4..7.  The LN chains for tile m run on DVE/ACT while the
        # PE works on later wo tiles, so the transposes never stall.
        late2 = ctx.enter_context(tc.tile_pool(name="late2", bufs=1))
        x1T = late2.tile([128, NDC, SHARD], BF16)
        x1keep = late2.tile([128, NQT, D], F32)
        h1T = late2.tile([128, NFC, SHARD], BF16)
        with tc.tile_pool(name="p3s", bufs=4) as p3s, \
             tc.tile_pool(name="p3st", bufs=4) as p3st, \
             tc.tile_pool(name="p5s", bufs=3) as p5s, \
             tc.tile_pool(name="p5st", bufs=4) as p5st, \
             tc.tile_pool(name="p4p", bufs=3, space="PSUM") as p4p:
            es3 = contextlib.ExitStack()
            p3p = es3.enter_context(tc.tile_pool(name="p3p", bufs=2, space="PSUM"))
            p3tp = es3.enter_context(tc.tile_pool(name="p3tp", bufs=2, space="PSUM"))

            def emit_wo_ln(m):
                ap_ = p3p.tile([128, D], F32, tag="apsum")
                for h in range(H):
                    nc.tensor.matmul(ap_, cn[:, h, m * 128:(m + 1) * 128],
                                     wo_sb[:, h, :], start=(h == 0),
                                     stop=(h == H - 1))
                xt = p3s.tile([128, D], F32, tag="xres")
                nc.sync.dma_start(out=xt, in_=xq[m * 128:(m + 1) * 128, :])
                t = p3s.tile([128, D], F32, tag="tres")
                nc.vector.tensor_add(out=t, in0=ap_, in1=xt)
                stats = p3st.tile([128, 6], F32, tag="stats")
                nc.vector.bn_stats(out=stats, in_=t)
                mv = p3st.tile([128, 2], F32, tag="mv")
                nc.vector.bn_aggr(out=mv, in_=stats)
                sd = p3st.tile([128, 1], F32, tag="sd")
                nc.scalar.activation(out=sd, in_=mv[:, 1:2], func=AF.Sqrt,
                                     bias=epst, scale=1.0)
                rs = p3st.tile([128, 1], F32, tag="rs")
                nc.vector.reciprocal_approx_fast(out=rs, in_=sd)
                x1m = x1keep[:, m, :]
                nc.vector.tensor_scalar(out=x1m, in0=t, scalar1=mv[:, 0:1],
                                        scalar2=rs, op0=ALU.subtract, op1=ALU.mult)
                if apply_affine1:
                    nc.vector.tensor_mul(out=x1m, in0=x1m, in1=g1b)
                    nc.vector.tensor_add(out=x1m, in0=x1m, in1=be1b)

            def emit_x1t(m):
                x1m = x1keep[:, m, :]
                tp = p3tp.tile([128, 512], F32, tag="tp3")
                for c in range(NDC):
                    nc.tensor.transpose(tp[:, c * 128:(c + 1) * 128],
                                        x1m[:, c * 128:(c + 1) * 128], ident)
                nc.scalar.copy(out=x1T[:, :, m * 128:(m + 1) * 128], in_=tp)

            def emit_ffn1(qh):
                for f in range(NFC):
                    hp = p4p.tile([128, 512], F32, tag="hpsum")
                    for c in range(NDC):
                        nc.tensor.matmul(hp,
                                         w1_sb[:, c, f * 128:(f + 1) * 128],
                                         x1T[:, c, qh * 512:(qh + 1) * 512],
                                         start=(c == 0), stop=(c == NDC - 1))
                    nc.scalar.activation(out=h1T[:, f, qh * 512:(qh + 1) * 512],
                                         in_=hp, func=AF.Relu,
                                         bias=b1_sb[:, f:f + 1], scale=1.0)

            def emit_ffn2(m, p5p):
                fp = p5p.tile([128, D], F32, tag="fpsum")
                for f in range(NFC):
                    nc.tensor.matmul(fp, h1T[:, f, m * 128:(m + 1) * 128],
                                     w2_sb[:, f, :], start=(f == 0),
                                     stop=(f == NFC - 1))
                t2 = p5s.tile([128, D], F32, tag="t2")
                nc.vector.tensor_add(out=t2, in0=fp, in1=x1keep[:, m, :])
                if apply_b2:
                    nc.vector.tensor_add(out=t2, in0=t2, in1=b2b)
                stats = p5st.tile([128, 6], F32, tag="stats5")
                nc.vector.bn_stats(out=stats, in_=t2)
                mv = p5st.tile([128, 2], F32, tag="mv5")
                nc.vector.bn_aggr(out=mv, in_=stats)
                sd = p5st.tile([128, 1], F32, tag="sd5")
                nc.scalar.activation(out=sd, in_=mv[:, 1:2], func=AF.Sqrt,
                                     bias=epst, scale=1.0)
                rs = p5st.tile([128, 1], F32, tag="rs5")
                nc.vector.reciprocal_approx_fast(out=rs, in_=sd)
                o = p5s.tile([128, D], F32, tag="otile")
                nc.vector.tensor_scalar(out=o, in0=t2, scalar1=mv[:, 0:1],
                                        scalar2=rs, op0=ALU.subtract, op1=ALU.mult)
                if apply_affine2:
                    nc.vector.tensor_mul(out=o, in0=o, in1=g2b)
                    nc.vector.tensor_add(out=o, in0=o, in1=be2b)
                nc.sync.dma_start(out=out[m * 128:(m + 1) * 128, :], in_=o)

            for m in range(NQT):
                emit_wo_ln(m)
            for m in range(4):
                emit_x1t(m)
            emit_ffn1(0)
            for m in range(4, NQT):
                emit_x1t(m)
            es3.close()  # free p3 psum pools before opening p5p
            with tc.tile_pool(name="p5p", bufs=2, space="PSUM") as p5p:
                for m in range(4):
                    emit_ffn2(m, p5p)
                emit_ffn1(1)
                for m in range(4, NQT):
                    emit_ffn2(m, p5p)

    nc.compile()
    return nc


_PROG_CACHE = {}


def _get_program(key):
    if key not in _PROG_CACHE:
        _PROG_CACHE[key] = _build_program(*key)
    return _PROG_CACHE[key]


def _make_in_maps(x, mask, wq, wk, wv, wo, w1, b1, w2, b2, g1, be1, g2, be2):
    f = np.float32
    bf = ml_dtypes.bfloat16
    # fold both the attention scale and the fast-exp scale EXA into wq;
    # scores then arrive as s*EXA and the exp trick is a single add.
    wq_s = (np.asarray(wq, f) * f(EXA / np.sqrt(DK))).astype(bf)
    mraw0 = np.where(np.asarray(mask)[:, 0, 0, :] == 0, f(-1e9), f(0.0))
    mtrick = mraw0 * f(EXA) + f(EXB)
    mraw = mraw0 + f(K8 * np.log(2.0))  # ACT path: fold the 2^K8 shift
    shared = dict(
        wq=wq_s, wk=np.asarray(wk, f).astype(bf), wv=np.asarray(wv, f).astype(bf),
        wo=np.asarray(wo, f).astype(bf), w1=np.asarray(w1, f).astype(bf),
        b1=np.ascontiguousarray(b1, f), w2=np.asarray(w2, f).astype(bf),
        b2=np.ascontiguousarray(b2, f), g1=np.ascontiguousarray(g1, f),
        be1=np.ascontiguousarray(be1, f), g2=np.ascontiguousarray(g2, f),
        be2=np.ascontiguousarray(be2, f),
    )
    in_maps = []
    for c in range(N_CORES):
        b, sh = c // 4, c % 4
        m = dict(shared)
        m["xf"] = np.ascontiguousarray(x[b], f)
        m["xq"] = np.ascontiguousarray(x[b, sh * SHARD:(sh + 1) * SHARD], f)
        m["mbr"] = np.ascontiguousarray(mraw[b], f)
        m["mbt"] = np.ascontiguousarray(mtrick[b], f)
        in_maps.append(m)
    return in_maps


def kernel(x, mask, wq, wk, wv, wo, w1, b1, w2, b2, g1, be1, g2, be2,
           _trace=False, _tmpdir=None):
    key = (
        not (np.all(np.asarray(g1) == 1.0) and np.all(np.asarray(be1) == 0.0)),
        not (np.all(np.asarray(g2) == 1.0) and np.all(np.asarray(be2) == 0.0)),
        not np.all(np.asarray(b2) == 0.0),
    )
    nc = _get_program(key)
    in_maps = _make_in_maps(x, mask, wq, wk, wv, wo, w1, b1, w2, b2,
                            g1, be1, g2, be2)
    res = None
    for attempt in range(3):
        try:
            res = run_bass_kernel_spmd(nc, in_maps, list(range(N_CORES)),
                                       trace=_trace, tmpdir=_tmpdir)
            break
        except Exception:
            if attempt == 2:
                raise
            import time as _time
            _time.sleep(2.0)
    outs = [res.results[c]["out"] for c in range(N_CORES)]
    full = np.empty((B, S, D), np.float32)
    for c in range(N_CORES):
        b, sh = c // 4, c % 4
        full[b, sh * SHARD:(sh + 1) * SHARD] = outs[c]
    kernel._last_results = res
    return full
